# revision 13
# baseline (speedup 1.0000x reference)
"""Trainium2 Bass kernel for nn_AtenMatmulQint8VM: dequantized int8-style
vector-matrix multiply  out = ((x - X_ZP)*X_SCALE) @ ((y - Y_ZP)*Y_SCALE).

Math: with xq = x - X_ZP and S = X_SCALE*Y_SCALE,
    out[n] = S * sum_k xq[k]*y[k,n]  -  S*Y_ZP * sum_k xq[k]
so y is only *cast* to bf16 (values 0..126 are exact in bf16) and the
y zero-point folds into a scalar bias computed from x on-device.

Distribution: y [8192,16384] int32 is sharded column-wise across 8 cores
(2048 cols each), x is replicated. Each core computes its 2048 outputs with
zero communication; the host concatenates the 8 shards.

Per-core kernel: y streams in 2-MiB chunks (2 K-tiles of [128,2048] int32)
via SWDGE DMA with an inline int32->bf16 cast — no on-chip dequant work.
TensorE accumulates the four 512-wide output slices as 4 column-tiled
matmuls (tile_position=(0,32q)) running concurrently in one PSUM bank,
so the vector-matrix multiply never bottlenecks on the cold-clock PE.
Epilogue applies scale and bias on VectorE. Measured 178.6 us/NEFF on HW
(~400 GB/s sustained HBM read per core; DMA-transfer-bound).
"""

import os
import sys

import numpy as np

sys.path.insert(0, "/opt/trn_rl_repo")

import concourse.bass as bass  # noqa: E402
import concourse.tile as tile  # noqa: E402
from concourse import bacc, mybir  # noqa: E402
from concourse.bass_utils import run_bass_kernel_spmd  # noqa: E402

X_SCALE, X_ZP = 0.0215, -25
Y_SCALE, Y_ZP = 0.0176, 18

K_FULL = 8192
N_FULL = 16384
NCORES = 8
P = 128
KT = K_FULL // P          # 64 K-tiles
N = N_FULL // NCORES      # 2048 output cols per core
NMM = 512                 # matmul free dim (one PSUM bank of fp32)

# Tunables (env-overridable for experiments)
DMA_CAST = os.environ.get("KQ_DMA_CAST", "1") == "1"
YBF_BUFS = int(os.environ.get("KQ_YBF_BUFS", "8"))
YI_BUFS = int(os.environ.get("KQ_YI_BUFS", "4"))
CHUNK = int(os.environ.get("KQ_CHUNK", "2"))      # K-tiles per DMA
COLTILE = os.environ.get("KQ_COLTILE", "1") == "1"  # 4x concurrent col-tiled MMs

TRACE = False          # set by test.py to capture a profile
LAST_RESULTS = None    # BassKernelResults of the last run when TRACE

_cache: dict = {}


def _build_nc():
    i32, f32, bf16 = mybir.dt.int32, mybir.dt.float32, mybir.dt.bfloat16
    S = X_SCALE * Y_SCALE

    nc = bacc.Bacc(
        "TRN2", target_bir_lowering=False, debug=False, num_devices=NCORES
    )
    x_dram = nc.dram_tensor("x_t", [P, KT], i32, kind="ExternalInput")
    y_dram = nc.dram_tensor("y", [K_FULL, N], i32, kind="ExternalInput")
    out_dram = nc.dram_tensor("out", [1, N], f32, kind="ExternalOutput")

    with tile.TileContext(nc) as tc:
        with (
            tc.tile_pool(name="xp", bufs=1) as xp,
            tc.tile_pool(name="yip", bufs=YI_BUFS) as yip,
            tc.tile_pool(name="ybfp", bufs=YBF_BUFS) as ybfp,
            tc.tile_pool(name="psp", bufs=1, space=bass.MemorySpace.PSUM) as psp,
            tc.tile_pool(name="op", bufs=1) as op,
        ):
            # ---- x: [P, KT] int32 (host-relaid column-major) -> xq bf16
            x_i = xp.tile([P, KT], i32)
            nc.sync.dma_start(x_i[:], x_dram[:])
            x_f = xp.tile([P, KT], f32)
            nc.vector.tensor_scalar_add(x_f[:], x_i[:], float(-X_ZP))
            x_bf = xp.tile([P, KT], bf16)
            nc.vector.tensor_copy(x_bf[:], x_f[:])

            # ---- bias = -S*Y_ZP * sum(xq), as [1, NQ] on partition 0
            NQ = N // NMM  # 4 col groups
            x_rowsum = xp.tile([P, NQ], f32)
            for q in range(NQ):
                nc.vector.tensor_reduce(
                    x_rowsum[:, q : q + 1],
                    x_f[:],
                    mybir.AxisListType.X,
                    mybir.AluOpType.add,
                )
            ones = xp.tile([P, 1], f32)
            nc.vector.memset(ones[:], 1.0)
            cx_ps = psp.tile([1, NQ], f32)
            nc.tensor.matmul(cx_ps[:], ones[:], x_rowsum[:], start=True, stop=True)
            bias = op.tile([1, NQ], f32)
            nc.vector.tensor_scalar_mul(bias[:], cx_ps[:], float(-S * Y_ZP))

            # ---- main loop over chunks of CHUNK K-tiles
            if COLTILE:
                # out row for col group q lives at PSUM partition 32q of one bank
                acc = psp.tile([P, NMM], f32)

                def acc_out(q):
                    return acc[32 * q : 32 * q + 1, :]

                def tile_pos(q):
                    return (0, 32 * q)
            else:
                acc = psp.tile([1, N], f32)

                def acc_out(q):
                    return acc[:, q * NMM : (q + 1) * NMM]

                def tile_pos(q):
                    return None

            # chunk schedule: CHUNK-sized transfers, except the last CHUNK
            # K-tiles go as single-tile chunks so the final matmul group only
            # waits on a small transfer (shorter kernel tail)
            if CHUNK > 1:
                sizes = [CHUNK] * (KT // CHUNK - 1) + [1] * CHUNK
            else:
                sizes = [1] * KT
            assert sum(sizes) == KT

            # [p, t, n] view: per-partition p, K-tile t, col n
            y_r = y_dram[:].rearrange("(t p) n -> p t n", p=P)
            t0 = 0
            for s in sizes:
                if DMA_CAST:
                    y_bf = ybfp.tile([P, CHUNK, N], bf16)
                    nc.gpsimd.dma_start(
                        y_bf[:, 0:s, :], y_r[:, t0 : t0 + s, :]
                    )  # inline int32->bf16
                else:
                    y_i = yip.tile([P, CHUNK, N], i32)
                    nc.sync.dma_start(y_i[:, 0:s, :], y_r[:, t0 : t0 + s, :])
                    y_bf = ybfp.tile([P, CHUNK, N], bf16)
                    if (t0 // CHUNK) % 2 == 0:
                        nc.vector.tensor_copy(y_bf[:, 0:s, :], y_i[:, 0:s, :])
                    else:
                        nc.scalar.copy(y_bf[:, 0:s, :], y_i[:, 0:s, :])
                for j in range(s):
                    t = t0 + j
                    for q in range(NQ):
                        nc.tensor.matmul(
                            acc_out(q),
                            x_bf[:, t : t + 1],
                            y_bf[:, j, q * NMM : (q + 1) * NMM],
                            start=(t == 0),
                            stop=(t == KT - 1),
                            tile_position=tile_pos(q),
                        )
                t0 += s

            # ---- epilogue: out = S*acc + bias
            if COLTILE:
                out_sb = op.tile([1, N], f32)
                for q in range(NQ):
                    nc.vector.tensor_scalar(
                        out_sb[0:1, q * NMM : (q + 1) * NMM],
                        acc[32 * q : 32 * q + 1, :],
                        float(S),
                        bias[0:1, q : q + 1],
                        mybir.AluOpType.mult,
                        mybir.AluOpType.add,
                    )
                nc.sync.dma_start(out_dram[:], out_sb[:])
            else:
                out_sb = op.tile([1, N], f32)
                nc.vector.tensor_scalar(
                    out_sb[:],
                    acc[:],
                    float(S),
                    bias[0:1, 0:1],
                    mybir.AluOpType.mult,
                    mybir.AluOpType.add,
                )
                nc.sync.dma_start(out_dram[:], out_sb[:])

    nc.compile()
    return nc


def kernel(x: np.ndarray, y: np.ndarray) -> np.ndarray:
    global LAST_RESULTS
    x = np.ascontiguousarray(np.asarray(x, dtype=np.int32))
    y = np.asarray(y, dtype=np.int32)
    assert x.shape == (K_FULL,) and y.shape == (K_FULL, N_FULL)

    if "nc" not in _cache:
        _cache["nc"] = _build_nc()
    nc = _cache["nc"]

    # host-side distribution: replicate x (relaid [P, KT] column-major so
    # K-tile t sits in SBUF column t), shard y column-wise
    x_t = np.ascontiguousarray(x.reshape(KT, P).T)
    in_maps = [
        {"x_t": x_t, "y": np.ascontiguousarray(y[:, i * N : (i + 1) * N])}
        for i in range(NCORES)
    ]

    res = run_bass_kernel_spmd(
        nc, in_maps, core_ids=list(range(NCORES)), trace=TRACE
    )
    LAST_RESULTS = res
    out = np.concatenate([r["out"].reshape(-1) for r in res.results])
    return out.astype(np.float32, copy=False)
